# revision 12
# baseline (speedup 1.0000x reference)
# kernel.py — self-contained Trainium2 Bass kernel for nn_BTDG_31774168055963 (moe_routing)
#
# Reference computation (see problem):
#   branch1: x1 = BN(S1[s]); pe1 = einsum('be,bef->bf', x1, (P1[p] @ G1.reshape(rd,ed*ed)).reshape(-1,ed,ed))
#            pe1 = BN(pe1); pred1 = pe1 @ O1.T
#   branch2: x2 = BN(S2[s]); m1 = x2*T_S[times]; per-coarse-bucket Tucker core G2[c]
#            pe2 = sum_c [c==fine2coarse[times]] einsum(m1, (P2[p] @ G2[c].reshape(rd,ed*ed)).reshape(-1,ed,ed))
#            pe2 = BN(pe2 * T_O[times]); pred2 = pe2 @ O2.T
#   out = sigmoid(pred1 + pred2)
#
# Strategy (8 NeuronCores):
#   - shard the Tucker rank dim rd=200 -> 25 per core (each core reads 1/8 of G1/G2, perfect balance)
#   - host sorts samples by coarse bucket (pure index routing); kernel does per-bucket matmuls
#   - Tucker contraction via outer-product moving operand Z^T[(r,e),b] = pp[r,b]*x[e,b]
#   - AllReduce the [400, 2048] partial pe across cores, BN on-device, logits matmul sharded
#     column-wise over E=20000 -> 2500 per core, sigmoid on ScalarE, host concatenates+unpermutes.
#   - all matmuls bf16 (fp32 PSUM accumulation); BN statistics fp32.

import numpy as np
import ml_dtypes

BF16 = ml_dtypes.bfloat16

B, E, R2, T, C, ED, RD = 2048, 20000, 500, 365, 12, 200, 200
NCORES = 8
RS = RD // NCORES       # 25 r's per core
ES = E // NCORES        # 2500 vocab per core
BN_EPS = 1e-5

_cache = {}


def _build(pieces, debug=False):
    """Build + compile the per-core bass kernel. `pieces` is a tuple of
    (coarse_id, col_off, col_len) for branch-2 bucket matmuls (512-grid aligned)."""
    import concourse.bass as bass
    import concourse.mybir as mybir
    import concourse.tile as tile
    from concourse import bacc

    f32 = mybir.dt.float32
    bf16 = mybir.dt.bfloat16

    nc = bacc.Bacc("TRN2", target_bir_lowering=False, debug=False, num_devices=NCORES)

    # ---------------- I/O ----------------
    x1_in = nc.dram_tensor("x1_in", [ED, B], bf16, kind="ExternalInput")   # S1[s_p].T
    x2_in = nc.dram_tensor("x2_in", [ED, B], bf16, kind="ExternalInput")   # S2[s_p].T
    ts_in = nc.dram_tensor("ts_in", [ED, B], bf16, kind="ExternalInput")   # T_S[times_p].T
    to_in = nc.dram_tensor("to_in", [ED, B], bf16, kind="ExternalInput")   # T_O[times_p].T
    g1_in = nc.dram_tensor("g1_in", [100, RS, 2, ED], bf16, kind="ExternalInput")
    g2_in = nc.dram_tensor("g2_in", [RS, 2, 100, C, ED], bf16, kind="ExternalInput")
    pb1_in = nc.dram_tensor("pb1_in", [RS, 100, B], bf16, kind="ExternalInput")  # host-replicated P1[p_p].T r-slice
    pb2_in = nc.dram_tensor("pb2_in", [RS, 100, B], bf16, kind="ExternalInput")
    # O chunks: feat layout {0:128, 128:200} x {O1, O2}
    oc0_in = nc.dram_tensor("oc0_in", [128, ES], bf16, kind="ExternalInput")
    oc1_in = nc.dram_tensor("oc1_in", [72, ES], bf16, kind="ExternalInput")
    oc2_in = nc.dram_tensor("oc2_in", [128, ES], bf16, kind="ExternalInput")
    oc3_in = nc.dram_tensor("oc3_in", [72, ES], bf16, kind="ExternalInput")
    bnp_in = nc.dram_tensor("bnp_in", [ED, 8], f32, kind="ExternalInput")  # g11,b11,g12,b12,g21,b21,g22,b22
    out_t = nc.dram_tensor("out", [B, ES], f32, kind="ExternalOutput")
    dbg_pe = dbg_x = None
    if debug:
        dbg_pe = nc.dram_tensor("dbg_pe", [2 * ED, B], f32, kind="ExternalOutput")
        dbg_x = nc.dram_tensor("dbg_x", [4, 100, B], bf16, kind="ExternalOutput")

    FS = [(0, 128), (128, 72)]  # feat M-tiles (offset, len)

    with tile.TileContext(nc) as tc:
        from contextlib import ExitStack
        with ExitStack() as ctx:
            singles = ctx.enter_context(tc.tile_pool(name="singles", bufs=1))
            xpool = ctx.enter_context(tc.tile_pool(name="xpool", bufs=1))
            small = ctx.enter_context(tc.tile_pool(name="small", bufs=4))
            btmp = ctx.enter_context(tc.tile_pool(name="btmp", bufs=2))
            perst = ctx.enter_context(tc.tile_pool(name="perst", bufs=1))
            dram = ctx.enter_context(tc.tile_pool(name="dram", bufs=1, space="DRAM"))

            # BN params in both partition alignments
            bnp100 = singles.tile([100, 2, 8], f32)
            nc.sync.dma_start(bnp100[:], bnp_in.rearrange("(h p) c -> p h c", p=100))
            bnpA = singles.tile([128, 8], f32)
            nc.sync.dma_start(bnpA[:], bnp_in[0:128, :])
            bnpB = singles.tile([72, 8], f32)
            nc.sync.dma_start(bnpB[:], bnp_in[128:200, :])
            eps100 = singles.tile([100, 1], f32)
            nc.vector.memset(eps100, BN_EPS)
            eps128 = singles.tile([128, 1], f32)
            nc.vector.memset(eps128, BN_EPS)

            def bn_normalize(src_ap, dst_tile, gcol, bcol, par_ap, eps_tile,
                             postmul=None, premul=None):
                """dst = BN(src [* premul]) * g + b [* postmul] — batch stats along free dim.
                src_ap/dst_tile: [P, B] tiles; gcol/bcol: columns in par_ap [P, ...].
                """
                P = dst_tile.shape[0]
                if premul is not None:
                    pre = btmp.tile([128, B], f32, tag="bn_pre")
                    nc.vector.tensor_tensor(pre[:P], src_ap, premul, mybir.AluOpType.mult)
                    src_ap = pre[:P]
                stats = small.tile([128, 4, 6], f32, tag="bn_stats")
                for i in range(4):
                    nc.vector.bn_stats(stats[:P, i, :], src_ap[:, i * 512:(i + 1) * 512])
                mv = small.tile([128, 2], f32, tag="bn_mv")
                nc.vector.bn_aggr(mv[:P], stats[:P])
                rstd = small.tile([128, 1], f32, tag="bn_rstd")
                nc.scalar.activation(rstd[:P], mv[:P, 1:2], mybir.ActivationFunctionType.Sqrt,
                                     bias=eps_tile[:P], scale=1.0)
                nc.vector.reciprocal(rstd[:P], rstd[:P])
                A = small.tile([128, 1], f32, tag="bn_A")
                nc.vector.tensor_mul(A[:P], rstd[:P], gcol)
                Bt = small.tile([128, 1], f32, tag="bn_B")
                nc.vector.tensor_mul(Bt[:P], mv[:P, 0:1], A[:P])
                nc.vector.tensor_tensor(Bt[:P], bcol, Bt[:P], mybir.AluOpType.subtract)
                nc.vector.tensor_scalar(dst_tile[:], src_ap, A[:P], Bt[:P],
                                        mybir.AluOpType.mult, mybir.AluOpType.add)
                if postmul is not None:
                    nc.vector.tensor_tensor(dst_tile[:], dst_tile[:], postmul,
                                            mybir.AluOpType.mult)

            # ---------- input BN ----------
            # x-side: feat layout in two [100, B] tiles (e-halves)
            x1t = []
            m1t = []
            for h in range(2):
                raw1 = btmp.tile([100, B], bf16, tag="raw_in")
                nc.sync.dma_start(raw1[:], x1_in[100 * h:100 * (h + 1), :])
                d1 = xpool.tile([100, B], bf16, name=f"x1t_{h}")
                bn_normalize(raw1[:], d1, bnp100[:, h, 0:1], bnp100[:, h, 1:2], bnp100, eps100)
                x1t.append(d1)

                raw2 = btmp.tile([100, B], bf16, tag="raw_in")
                nc.sync.dma_start(raw2[:], x2_in[100 * h:100 * (h + 1), :])
                tsh = btmp.tile([100, B], bf16, tag="ts_in")
                nc.sync.dma_start(tsh[:], ts_in[100 * h:100 * (h + 1), :])
                d2 = xpool.tile([100, B], bf16, name=f"m1t_{h}")
                bn_normalize(raw2[:], d2, bnp100[:, h, 4:5], bnp100[:, h, 5:6], bnp100, eps100,
                             postmul=tsh[:])
                m1t.append(d2)

            # ---------- Tucker branches ----------
            pe_dram = dram.tile([2 * ED, B], f32)      # AllReduce input bounce
            pe_out_dram = dram.tile([2 * ED, B], f32, addr_space="Shared")

            with tc.tile_pool(name="tucker", bufs=3) as tpool, \
                 tc.tile_pool(name="gw", bufs=3) as gwpool, \
                 tc.tile_pool(name="psum_tk", bufs=1, space="PSUM") as pst:

                g1_sb = singles.tile([100, RS, 2, ED], bf16)
                for r5 in range(5):
                    nc.sync.dma_start(
                        g1_sb[:, r5 * 5:(r5 + 1) * 5],
                        g1_in[:, r5 * 5:(r5 + 1) * 5])

                # ----- branch 1: full-batch 512 chunks -----
                ps1_a = pst.tile([128, B], f32, tag="ps_m0", name="ps1_a")
                ps1_b = pst.tile([72, B], f32, tag="ps_m1", name="ps1_b")
                ps1 = [ps1_a, ps1_b]
                for r in range(RS):
                    pb = tpool.tile([100, B], bf16, tag="ppb")
                    nc.sync.dma_start(pb[:], pb1_in[r])
                    for h in range(2):
                        z = tpool.tile([100, B], bf16, tag="z")
                        nc.vector.tensor_tensor(z[:], x1t[h][:], pb[:], mybir.AluOpType.mult)
                        first = (r == 0 and h == 0)
                        last = (r == RS - 1 and h == 1)
                        for mi, (mo, ml) in enumerate(FS):
                            for bc in range(4):
                                nc.tensor.matmul(
                                    ps1[mi][:, bc * 512:(bc + 1) * 512],
                                    lhsT=g1_sb[:, r, h, mo:mo + ml],
                                    rhs=z[:, bc * 512:(bc + 1) * 512],
                                    start=first, stop=last)
                # evict branch 1 -> DRAM bounce (rows 0:200)
                for mi, (mo, ml) in enumerate(FS):
                    pe_sb = btmp.tile([128, B], f32, tag="pe_evict")
                    nc.vector.tensor_copy(pe_sb[:ml], ps1[mi][:])
                    nc.sync.dma_start(pe_dram[mo:mo + ml, :], pe_sb[:ml])

                # ----- branch 2: per-bucket pieces -----
                ps2_a = pst.tile([128, B], f32, tag="ps_m0", name="ps2_a")
                ps2_b = pst.tile([72, B], f32, tag="ps_m1", name="ps2_b")
                ps2 = [ps2_a, ps2_b]
                for r in range(RS):
                    pb = tpool.tile([100, B], bf16, tag="ppb")
                    nc.sync.dma_start(pb[:], pb2_in[r])
                    for h in range(2):
                        g2c = gwpool.tile([100, C, ED], bf16, tag="g2w")
                        nc.sync.dma_start(g2c[:], g2_in[r, h])
                        z = tpool.tile([100, B], bf16, tag="z")
                        nc.vector.tensor_tensor(z[:], m1t[h][:], pb[:], mybir.AluOpType.mult)
                        first = (r == 0 and h == 0)
                        last = (r == RS - 1 and h == 1)
                        for mi, (mo, ml) in enumerate(FS):
                            seen_banks = set()
                            for (cid, off, ln) in pieces:
                                bank = off // 512
                                bank_first = bank not in seen_banks
                                seen_banks.add(bank)
                                nc.tensor.matmul(
                                    ps2[mi][:, off:off + ln],
                                    lhsT=g2c[:, cid, mo:mo + ml],
                                    rhs=z[:, off:off + ln],
                                    start=(first and bank_first), stop=last,
                                    skip_group_check=True)
                for mi, (mo, ml) in enumerate(FS):
                    pe_sb = btmp.tile([128, B], f32, tag="pe_evict")
                    nc.vector.tensor_copy(pe_sb[:ml], ps2[mi][:])
                    nc.sync.dma_start(pe_dram[ED + mo:ED + mo + ml, :], pe_sb[:ml])

            # ---------- AllReduce partial pe across the 8 cores ----------
            nc.gpsimd.collective_compute(
                "AllReduce", mybir.AluOpType.add,
                replica_groups=[list(range(NCORES))],
                ins=[pe_dram.opt()], outs=[pe_out_dram.opt()])

            if debug:
                nc.sync.dma_start(dbg_pe[:, :], pe_out_dram[:, :])
                for h in range(2):
                    nc.sync.dma_start(dbg_x[0 + h], x1t[h][:])
                    nc.sync.dma_start(dbg_x[2 + h], m1t[h][:])

            # ---------- post BN + logits ----------
            with tc.tile_pool(name="logits", bufs=4) as lpool, \
                 tc.tile_pool(name="ocat", bufs=1) as opool, \
                 tc.tile_pool(name="psum_l", bufs=4, space="PSUM") as psl:

                oc_sb = []
                for i, (oin, P) in enumerate([(oc0_in, 128), (oc1_in, 72), (oc2_in, 128), (oc3_in, 72)]):
                    t = opool.tile([P, ES], bf16, name=f"oc_{i}")
                    nc.sync.dma_start(t[:], oin[:])
                    oc_sb.append(t)

                # read back reduced pe; branch2 extra T_O multiply; BN with g12/b12, g22/b22
                pe_bn = []
                for br in range(2):
                    for mi, (mo, ml) in enumerate(FS):
                        raw = btmp.tile([128, B], f32, tag="pe_raw")
                        nc.sync.dma_start(raw[:ml], pe_out_dram[ED * br + mo:ED * br + mo + ml, :])
                        extra = None
                        if br == 1:
                            toh = btmp.tile([128, B], bf16, tag="to_in")
                            nc.sync.dma_start(toh[:ml], to_in[mo:mo + ml, :])
                            extra = toh[:ml]
                        par = bnpA if mi == 0 else bnpB
                        dst = perst.tile([128, B], bf16, name=f"pebn_{br}_{mi}")
                        gcol = par[:, 2 + 4 * br:3 + 4 * br]
                        bcol = par[:, 3 + 4 * br:4 + 4 * br]
                        bn_normalize(raw[:ml], dst[:ml], gcol, bcol, par, eps128, premul=extra)
                        pe_bn.append((dst, ml))

                # logits matmul + sigmoid + store
                for mb in range(B // 128):
                    for vc in range(ES // 500):
                        psu = psl.tile([128, 512], f32, tag="ps_l")
                        for kc in range(4):
                            peb, kl = pe_bn[kc]
                            nc.tensor.matmul(
                                psu[:, 0:500],
                                lhsT=peb[:kl, mb * 128:(mb + 1) * 128],
                                rhs=oc_sb[kc][:, vc * 500:(vc + 1) * 500],
                                start=(kc == 0), stop=(kc == 3))
                        so = lpool.tile([128, 500], f32, tag="sigout")
                        nc.scalar.activation(so[:], psu[:, 0:500],
                                             mybir.ActivationFunctionType.Sigmoid)
                        nc.sync.dma_start(out_t[mb * 128:(mb + 1) * 128, vc * 500:(vc + 1) * 500], so[:])

    nc.compile()
    return nc


def kernel(s, p, o, times, fine2coarse, S1, O1, S2, O2, P1, P2, G1, G2, T_S, T_O,
           g11, b11, g12, b12, g21, b21, g22, b22):
    from concourse.bass_utils import run_bass_kernel_spmd

    s = np.asarray(s); p = np.asarray(p); times = np.asarray(times)
    fine2coarse = np.asarray(fine2coarse)

    # ----- host-side routing (index logistics only) -----
    c = fine2coarse[times]                       # [B] coarse id per sample
    perm = np.argsort(c, kind="stable")
    c_sorted = c[perm]
    counts = np.bincount(c_sorted, minlength=C)
    offs = np.concatenate([[0], np.cumsum(counts)])
    pieces = []
    for cid in range(C):
        pos, en = int(offs[cid]), int(offs[cid + 1])
        while pos < en:
            nxt = min(en, (pos // 512 + 1) * 512)
            pieces.append((cid, pos, nxt - pos))
            pos = nxt
    pieces = tuple(pieces)

    key = pieces
    if key not in _cache:
        _cache[key] = _build(pieces)
    nc = _cache[key]

    s_p, p_p, t_p = s[perm], p[perm], times[perm]

    def bt(x):
        return np.ascontiguousarray(x, dtype=BF16)

    x1_in = bt(np.asarray(S1)[s_p].T)
    x2_in = bt(np.asarray(S2)[s_p].T)
    ts_in = bt(np.asarray(T_S)[t_p].T)
    to_in = bt(np.asarray(T_O)[t_p].T)
    pp1 = np.asarray(P1)[p_p]                       # [B, RD]
    pp2 = np.asarray(P2)[p_p]
    G1 = np.asarray(G1); G2 = np.asarray(G2)
    O1 = np.asarray(O1); O2 = np.asarray(O2)
    bnp = np.stack([g11, b11, g12, b12, g21, b21, g22, b22], axis=1).astype(np.float32)
    bnp = np.ascontiguousarray(bnp)

    in_maps = []
    for k in range(NCORES):
        rs = slice(RS * k, RS * (k + 1))
        vs = slice(ES * k, ES * (k + 1))
        g1k = bt(G1[rs].reshape(RS, 2, 100, ED).transpose(2, 0, 1, 3))
        g2k = bt(G2[:, rs].reshape(C, RS, 2, 100, ED).transpose(1, 2, 3, 0, 4))
        pb1 = bt(np.broadcast_to(pp1[:, rs].T[:, None, :], (RS, 100, B)))
        pb2 = bt(np.broadcast_to(pp2[:, rs].T[:, None, :], (RS, 100, B)))
        o1t = bt(O1[vs].T)   # [200, ES]
        o2t = bt(O2[vs].T)
        in_maps.append({
            "x1_in": x1_in, "x2_in": x2_in, "ts_in": ts_in, "to_in": to_in,
            "g1_in": g1k, "g2_in": g2k, "pb1_in": pb1, "pb2_in": pb2,
            "oc0_in": np.ascontiguousarray(o1t[0:128]),
            "oc1_in": np.ascontiguousarray(o1t[128:200]),
            "oc2_in": np.ascontiguousarray(o2t[0:128]),
            "oc3_in": np.ascontiguousarray(o2t[128:200]),
            "bnp_in": bnp,
        })

    res = run_bass_kernel_spmd(nc, in_maps, core_ids=list(range(NCORES)))

    out_sorted = np.concatenate([res.results[k]["out"] for k in range(NCORES)], axis=1)
    out = np.empty_like(out_sorted)
    out[perm] = out_sorted
    return out


# revision 13
# speedup vs baseline: 27243.7375x; 27243.7375x over previous
# kernel.py — self-contained Trainium2 Bass kernel for nn_BTDG_31774168055963 (moe_routing)
#
# Reference computation (see problem):
#   branch1: x1 = BN(S1[s]); pe1 = einsum('be,bef->bf', x1, (P1[p] @ G1.reshape(rd,ed*ed)).reshape(-1,ed,ed))
#            pe1 = BN(pe1); pred1 = pe1 @ O1.T
#   branch2: x2 = BN(S2[s]); m1 = x2*T_S[times]; per-coarse-bucket Tucker core G2[c]
#            pe2 = sum_c [c==fine2coarse[times]] einsum(m1, (P2[p] @ G2[c].reshape(rd,ed*ed)).reshape(-1,ed,ed))
#            pe2 = BN(pe2 * T_O[times]); pred2 = pe2 @ O2.T
#   out = sigmoid(pred1 + pred2)
#
# Strategy (8 NeuronCores):
#   - shard the Tucker rank dim rd=200 -> 25 per core (each core reads 1/8 of G1/G2, perfect balance)
#   - host sorts samples by coarse bucket (pure index routing); kernel does per-bucket matmuls
#   - Tucker contraction via outer-product moving operand Z^T[(r,e),b] = pp[r,b]*x[e,b]
#   - AllReduce the [400, 2048] partial pe across cores, BN on-device, logits matmul sharded
#     column-wise over E=20000 -> 2500 per core, sigmoid on ScalarE, host concatenates+unpermutes.
#   - all matmuls bf16 (fp32 PSUM accumulation); BN statistics fp32.

import numpy as np
import ml_dtypes

BF16 = ml_dtypes.bfloat16

B, E, R2, T, C, ED, RD = 2048, 20000, 500, 365, 12, 200, 200
NCORES = 8
RS = RD // NCORES       # 25 r's per core
ES = E // NCORES        # 2500 vocab per core
BN_EPS = 1e-5

_cache = {}


def _build(pieces, debug=False):
    """Build + compile the per-core bass kernel. `pieces` is a tuple of
    (coarse_id, col_off, col_len) for branch-2 bucket matmuls (512-grid aligned)."""
    import concourse.bass as bass
    import concourse.mybir as mybir
    import concourse.tile as tile
    from concourse import bacc

    f32 = mybir.dt.float32
    bf16 = mybir.dt.bfloat16

    nc = bacc.Bacc("TRN2", target_bir_lowering=False, debug=False, num_devices=NCORES)

    # ---------------- I/O ----------------
    x1_in = nc.dram_tensor("x1_in", [ED, B], bf16, kind="ExternalInput")   # S1[s_p].T
    x2_in = nc.dram_tensor("x2_in", [ED, B], bf16, kind="ExternalInput")   # S2[s_p].T
    ts_in = nc.dram_tensor("ts_in", [ED, B], bf16, kind="ExternalInput")   # T_S[times_p].T
    to_in = nc.dram_tensor("to_in", [ED, B], bf16, kind="ExternalInput")   # T_O[times_p].T
    g1_in = nc.dram_tensor("g1_in", [100, RS, 2, ED], bf16, kind="ExternalInput")
    g2_in = nc.dram_tensor("g2_in", [RS, 2, 100, C, ED], bf16, kind="ExternalInput")
    pb1_in = nc.dram_tensor("pb1_in", [RS, B], bf16, kind="ExternalInput")  # P1[p_p].T r-slice
    pb2_in = nc.dram_tensor("pb2_in", [RS, B], bf16, kind="ExternalInput")
    # O chunks: feat layout {0:128, 128:200} x {O1, O2}
    oc0_in = nc.dram_tensor("oc0_in", [128, ES], bf16, kind="ExternalInput")
    oc1_in = nc.dram_tensor("oc1_in", [72, ES], bf16, kind="ExternalInput")
    oc2_in = nc.dram_tensor("oc2_in", [128, ES], bf16, kind="ExternalInput")
    oc3_in = nc.dram_tensor("oc3_in", [72, ES], bf16, kind="ExternalInput")
    bnp_in = nc.dram_tensor("bnp_in", [ED, 8], f32, kind="ExternalInput")  # g11,b11,g12,b12,g21,b21,g22,b22
    out_t = nc.dram_tensor("out", [B, ES], f32, kind="ExternalOutput")
    dbg_pe = dbg_x = None
    if debug:
        dbg_pe = nc.dram_tensor("dbg_pe", [2 * ED, B], f32, kind="ExternalOutput")
        dbg_x = nc.dram_tensor("dbg_x", [4, 100, B], bf16, kind="ExternalOutput")

    FS = [(0, 128), (128, 72)]  # feat M-tiles (offset, len)

    with tile.TileContext(nc) as tc:
        from contextlib import ExitStack
        with ExitStack() as ctx:
            singles = ctx.enter_context(tc.tile_pool(name="singles", bufs=1))
            from concourse import library_config
            nc.gpsimd.load_library(library_config.attn)
            xpool = ctx.enter_context(tc.tile_pool(name="xpool", bufs=1))
            small = ctx.enter_context(tc.tile_pool(name="small", bufs=4))
            btmp = ctx.enter_context(tc.tile_pool(name="btmp", bufs=2))
            perst = ctx.enter_context(tc.tile_pool(name="perst", bufs=1))
            dram = ctx.enter_context(tc.tile_pool(name="dram", bufs=1, space="DRAM"))

            # BN params in both partition alignments
            bnp100 = singles.tile([100, 2, 8], f32)
            nc.sync.dma_start(bnp100[:], bnp_in.rearrange("(h p) c -> p h c", p=100))
            bnpA = singles.tile([128, 8], f32)
            nc.sync.dma_start(bnpA[:], bnp_in[0:128, :])
            bnpB = singles.tile([72, 8], f32)
            nc.sync.dma_start(bnpB[:], bnp_in[128:200, :])
            eps100 = singles.tile([100, 1], f32)
            nc.vector.memset(eps100, BN_EPS)
            eps128 = singles.tile([128, 1], f32)
            nc.vector.memset(eps128, BN_EPS)

            def bn_normalize(src_ap, dst_tile, gcol, bcol, par_ap, eps_tile,
                             postmul=None, premul=None):
                """dst = BN(src [* premul]) * g + b [* postmul] — batch stats along free dim.
                src_ap/dst_tile: [P, B] tiles; gcol/bcol: columns in par_ap [P, ...].
                """
                P = dst_tile.shape[0]
                if premul is not None:
                    pre = btmp.tile([128, B], f32, tag="bn_pre")
                    nc.vector.tensor_tensor(pre[:P], src_ap, premul, mybir.AluOpType.mult)
                    src_ap = pre[:P]
                stats = small.tile([128, 4, 6], f32, tag="bn_stats")
                for i in range(4):
                    nc.vector.bn_stats(stats[:P, i, :], src_ap[:, i * 512:(i + 1) * 512])
                mv = small.tile([128, 2], f32, tag="bn_mv")
                nc.vector.bn_aggr(mv[:P], stats[:P])
                rstd = small.tile([128, 1], f32, tag="bn_rstd")
                nc.scalar.activation(rstd[:P], mv[:P, 1:2], mybir.ActivationFunctionType.Sqrt,
                                     bias=eps_tile[:P], scale=1.0)
                nc.vector.reciprocal(rstd[:P], rstd[:P])
                A = small.tile([128, 1], f32, tag="bn_A")
                nc.vector.tensor_mul(A[:P], rstd[:P], gcol)
                Bt = small.tile([128, 1], f32, tag="bn_B")
                nc.vector.tensor_mul(Bt[:P], mv[:P, 0:1], A[:P])
                nc.vector.tensor_tensor(Bt[:P], bcol, Bt[:P], mybir.AluOpType.subtract)
                nc.vector.tensor_scalar(dst_tile[:], src_ap, A[:P], Bt[:P],
                                        mybir.AluOpType.mult, mybir.AluOpType.add)
                if postmul is not None:
                    nc.vector.tensor_tensor(dst_tile[:], dst_tile[:], postmul,
                                            mybir.AluOpType.mult)

            # ---------- input BN ----------
            # x-side: feat layout in two [100, B] tiles (e-halves)
            x1t = []
            m1t = []
            for h in range(2):
                raw1 = btmp.tile([100, B], bf16, tag="raw_in")
                nc.sync.dma_start(raw1[:], x1_in[100 * h:100 * (h + 1), :])
                d1 = xpool.tile([100, B], bf16, name=f"x1t_{h}")
                bn_normalize(raw1[:], d1, bnp100[:, h, 0:1], bnp100[:, h, 1:2], bnp100, eps100)
                x1t.append(d1)

                raw2 = btmp.tile([100, B], bf16, tag="raw_in")
                nc.sync.dma_start(raw2[:], x2_in[100 * h:100 * (h + 1), :])
                tsh = btmp.tile([100, B], bf16, tag="ts_in")
                nc.sync.dma_start(tsh[:], ts_in[100 * h:100 * (h + 1), :])
                d2 = xpool.tile([100, B], bf16, name=f"m1t_{h}")
                bn_normalize(raw2[:], d2, bnp100[:, h, 4:5], bnp100[:, h, 5:6], bnp100, eps100,
                             postmul=tsh[:])
                m1t.append(d2)

            # ---------- Tucker branches ----------
            pe1_dram = dram.tile([ED, B], f32)
            pe1_out_dram = dram.tile([ED, B], f32, addr_space="Shared")
            pe2_dram = dram.tile([ED, B], f32)
            pe2_out_dram = dram.tile([ED, B], f32, addr_space="Shared")

            with tc.tile_pool(name="tucker", bufs=3) as tpool, \
                 tc.tile_pool(name="gw", bufs=3) as gwpool, \
                 tc.tile_pool(name="psum_tk", bufs=1, space="PSUM") as pst:

                g1_sb = singles.tile([100, RS, 2, ED], bf16)
                for r5 in range(5):
                    nc.sync.dma_start(
                        g1_sb[:, r5 * 5:(r5 + 1) * 5],
                        g1_in[:, r5 * 5:(r5 + 1) * 5])

                # ----- branch 1: full-batch 512 chunks -----
                ps1_a = pst.tile([128, B], f32, tag="ps_m0", name="ps1_a")
                ps1_b = pst.tile([72, B], f32, tag="ps_m1", name="ps1_b")
                ps1 = [ps1_a, ps1_b]
                for r in range(RS):
                    pbs = tpool.tile([1, B], bf16, tag="ppbsrc")
                    nc.sync.dma_start(pbs[:], pb1_in[r:r + 1, :])
                    pb = tpool.tile([100, B], bf16, tag="ppb")
                    nc.gpsimd.partition_broadcast(pb[:], pbs[:])
                    for h in range(2):
                        z = tpool.tile([100, B], bf16, tag="z")
                        nc.vector.tensor_tensor(z[:], x1t[h][:], pb[:], mybir.AluOpType.mult)
                        first = (r == 0 and h == 0)
                        last = (r == RS - 1 and h == 1)
                        for mi, (mo, ml) in enumerate(FS):
                            for bc in range(4):
                                nc.tensor.matmul(
                                    ps1[mi][:, bc * 512:(bc + 1) * 512],
                                    lhsT=g1_sb[:, r, h, mo:mo + ml],
                                    rhs=z[:, bc * 512:(bc + 1) * 512],
                                    start=first, stop=last)
                # evict branch 1 -> DRAM bounce (rows 0:200)
                for mi, (mo, ml) in enumerate(FS):
                    pe_sb = btmp.tile([128, B], f32, tag="pe_evict")
                    nc.vector.tensor_copy(pe_sb[:ml], ps1[mi][:])
                    nc.sync.dma_start(pe1_dram[mo:mo + ml, :], pe_sb[:ml])
                nc.gpsimd.collective_compute(
                    "AllReduce", mybir.AluOpType.add,
                    replica_groups=[list(range(NCORES))],
                    ins=[pe1_dram.opt()], outs=[pe1_out_dram.opt()])

                # ----- branch 2: per-bucket pieces -----
                ps2_a = pst.tile([128, B], f32, tag="ps_m0", name="ps2_a")
                ps2_b = pst.tile([72, B], f32, tag="ps_m1", name="ps2_b")
                ps2 = [ps2_a, ps2_b]
                for r in range(RS):
                    pbs = tpool.tile([1, B], bf16, tag="ppbsrc")
                    nc.sync.dma_start(pbs[:], pb2_in[r:r + 1, :])
                    pb = tpool.tile([100, B], bf16, tag="ppb")
                    nc.gpsimd.partition_broadcast(pb[:], pbs[:])
                    for h in range(2):
                        g2c = gwpool.tile([100, C, ED], bf16, tag="g2w")
                        nc.sync.dma_start(g2c[:], g2_in[r, h])
                        z = tpool.tile([100, B], bf16, tag="z")
                        nc.vector.tensor_tensor(z[:], m1t[h][:], pb[:], mybir.AluOpType.mult)
                        first = (r == 0 and h == 0)
                        last = (r == RS - 1 and h == 1)
                        for mi, (mo, ml) in enumerate(FS):
                            seen_banks = set()
                            for (cid, off, ln) in pieces:
                                bank = off // 512
                                bank_first = bank not in seen_banks
                                seen_banks.add(bank)
                                nc.tensor.matmul(
                                    ps2[mi][:, off:off + ln],
                                    lhsT=g2c[:, cid, mo:mo + ml],
                                    rhs=z[:, off:off + ln],
                                    start=(first and bank_first), stop=last,
                                    skip_group_check=True)
                for mi, (mo, ml) in enumerate(FS):
                    pe_sb = btmp.tile([128, B], f32, tag="pe_evict")
                    nc.vector.tensor_copy(pe_sb[:ml], ps2[mi][:])
                    nc.sync.dma_start(pe2_dram[mo:mo + ml, :], pe_sb[:ml])

            # ---------- AllReduce pe2 (pe1's AR already overlaps branch 2) ----------
            nc.gpsimd.collective_compute(
                "AllReduce", mybir.AluOpType.add,
                replica_groups=[list(range(NCORES))],
                ins=[pe2_dram.opt()], outs=[pe2_out_dram.opt()])

            if debug:
                nc.sync.dma_start(dbg_pe[0:ED, :], pe1_out_dram[:, :])
                nc.sync.dma_start(dbg_pe[ED:2 * ED, :], pe2_out_dram[:, :])
                for h in range(2):
                    nc.sync.dma_start(dbg_x[0 + h], x1t[h][:])
                    nc.sync.dma_start(dbg_x[2 + h], m1t[h][:])

            # ---------- post BN + logits ----------
            with tc.tile_pool(name="logits", bufs=4) as lpool, \
                 tc.tile_pool(name="ocat", bufs=1) as opool, \
                 tc.tile_pool(name="psum_l", bufs=4, space="PSUM") as psl:

                oc_sb = []
                for i, (oin, P) in enumerate([(oc0_in, 128), (oc1_in, 72), (oc2_in, 128), (oc3_in, 72)]):
                    t = opool.tile([P, ES], bf16, name=f"oc_{i}")
                    nc.sync.dma_start(t[:], oin[:])
                    oc_sb.append(t)

                # read back reduced pe; branch2 extra T_O multiply; BN with g12/b12, g22/b22
                pe_bn = []
                for br in range(2):
                    for mi, (mo, ml) in enumerate(FS):
                        raw = btmp.tile([128, B], f32, tag="pe_raw")
                        ped = pe1_out_dram if br == 0 else pe2_out_dram
                        nc.sync.dma_start(raw[:ml], ped[mo:mo + ml, :])
                        extra = None
                        if br == 1:
                            toh = btmp.tile([128, B], bf16, tag="to_in")
                            nc.sync.dma_start(toh[:ml], to_in[mo:mo + ml, :])
                            extra = toh[:ml]
                        par = bnpA if mi == 0 else bnpB
                        dst = perst.tile([128, B], bf16, name=f"pebn_{br}_{mi}")
                        gcol = par[:, 2 + 4 * br:3 + 4 * br]
                        bcol = par[:, 3 + 4 * br:4 + 4 * br]
                        bn_normalize(raw[:ml], dst[:ml], gcol, bcol, par, eps128, premul=extra)
                        pe_bn.append((dst, ml))

                # logits matmul + sigmoid + store
                for mb in range(B // 128):
                    orow = lpool.tile([128, ES], f32, tag="orow", bufs=2)
                    for vc in range(ES // 500):
                        psu = psl.tile([128, 512], f32, tag="ps_l")
                        for kc in range(4):
                            peb, kl = pe_bn[kc]
                            nc.tensor.matmul(
                                psu[:, 0:500],
                                lhsT=peb[:kl, mb * 128:(mb + 1) * 128],
                                rhs=oc_sb[kc][:, vc * 500:(vc + 1) * 500],
                                start=(kc == 0), stop=(kc == 3))
                        nc.scalar.activation(orow[:, vc * 500:(vc + 1) * 500], psu[:, 0:500],
                                             mybir.ActivationFunctionType.Sigmoid)
                    nc.sync.dma_start(out_t[mb * 128:(mb + 1) * 128, :], orow[:])

    nc.compile()
    return nc


def kernel(s, p, o, times, fine2coarse, S1, O1, S2, O2, P1, P2, G1, G2, T_S, T_O,
           g11, b11, g12, b12, g21, b21, g22, b22):
    from concourse.bass_utils import run_bass_kernel_spmd

    s = np.asarray(s); p = np.asarray(p); times = np.asarray(times)
    fine2coarse = np.asarray(fine2coarse)

    # ----- host-side routing (index logistics only) -----
    c = fine2coarse[times]                       # [B] coarse id per sample
    perm = np.argsort(c, kind="stable")
    c_sorted = c[perm]
    counts = np.bincount(c_sorted, minlength=C)
    offs = np.concatenate([[0], np.cumsum(counts)])
    pieces = []
    for cid in range(C):
        pos, en = int(offs[cid]), int(offs[cid + 1])
        while pos < en:
            nxt = min(en, (pos // 512 + 1) * 512)
            pieces.append((cid, pos, nxt - pos))
            pos = nxt
    pieces = tuple(pieces)

    key = pieces
    if key not in _cache:
        _cache[key] = _build(pieces)
    nc = _cache[key]

    s_p, p_p, t_p = s[perm], p[perm], times[perm]

    def bt(x):
        return np.ascontiguousarray(x, dtype=BF16)

    x1_in = bt(np.asarray(S1)[s_p].T)
    x2_in = bt(np.asarray(S2)[s_p].T)
    ts_in = bt(np.asarray(T_S)[t_p].T)
    to_in = bt(np.asarray(T_O)[t_p].T)
    pp1 = np.asarray(P1)[p_p]                       # [B, RD]
    pp2 = np.asarray(P2)[p_p]
    G1 = np.asarray(G1); G2 = np.asarray(G2)
    O1 = np.asarray(O1); O2 = np.asarray(O2)
    bnp = np.stack([g11, b11, g12, b12, g21, b21, g22, b22], axis=1).astype(np.float32)
    bnp = np.ascontiguousarray(bnp)

    in_maps = []
    for k in range(NCORES):
        rs = slice(RS * k, RS * (k + 1))
        vs = slice(ES * k, ES * (k + 1))
        g1k = bt(G1[rs].reshape(RS, 2, 100, ED).transpose(2, 0, 1, 3))
        g2k = bt(G2[:, rs].reshape(C, RS, 2, 100, ED).transpose(1, 2, 3, 0, 4))
        pb1 = bt(pp1[:, rs].T)
        pb2 = bt(pp2[:, rs].T)
        o1t = bt(O1[vs].T)   # [200, ES]
        o2t = bt(O2[vs].T)
        in_maps.append({
            "x1_in": x1_in, "x2_in": x2_in, "ts_in": ts_in, "to_in": to_in,
            "g1_in": g1k, "g2_in": g2k, "pb1_in": pb1, "pb2_in": pb2,
            "oc0_in": np.ascontiguousarray(o1t[0:128]),
            "oc1_in": np.ascontiguousarray(o1t[128:200]),
            "oc2_in": np.ascontiguousarray(o2t[0:128]),
            "oc3_in": np.ascontiguousarray(o2t[128:200]),
            "bnp_in": bnp,
        })

    res = run_bass_kernel_spmd(nc, in_maps, core_ids=list(range(NCORES)))

    out_sorted = np.concatenate([res.results[k]["out"] for k in range(NCORES)], axis=1)
    out = np.empty_like(out_sorted)
    out[perm] = out_sorted
    return out


# revision 14
# speedup vs baseline: 30040.4838x; 1.1027x over previous
# kernel.py — self-contained Trainium2 Bass kernel for nn_BTDG_31774168055963 (moe_routing)
#
# Reference computation (see problem):
#   branch1: x1 = BN(S1[s]); pe1 = einsum('be,bef->bf', x1, (P1[p] @ G1.reshape(rd,ed*ed)).reshape(-1,ed,ed))
#            pe1 = BN(pe1); pred1 = pe1 @ O1.T
#   branch2: x2 = BN(S2[s]); m1 = x2*T_S[times]; per-coarse-bucket Tucker core G2[c]
#            pe2 = sum_c [c==fine2coarse[times]] einsum(m1, (P2[p] @ G2[c].reshape(rd,ed*ed)).reshape(-1,ed,ed))
#            pe2 = BN(pe2 * T_O[times]); pred2 = pe2 @ O2.T
#   out = sigmoid(pred1 + pred2)
#
# Strategy (8 NeuronCores):
#   - shard the Tucker rank dim rd=200 -> 25 per core (each core reads 1/8 of G1/G2, perfect balance)
#   - host sorts samples by coarse bucket (pure index routing); kernel does per-bucket matmuls
#   - Tucker contraction via outer-product moving operand Z^T[(r,e),b] = pp[r,b]*x[e,b]
#   - AllReduce the [400, 2048] partial pe across cores, BN on-device, logits matmul sharded
#     column-wise over E=20000 -> 2500 per core, sigmoid on ScalarE, host concatenates+unpermutes.
#   - all matmuls bf16 (fp32 PSUM accumulation); BN statistics fp32.

import numpy as np
import ml_dtypes

BF16 = ml_dtypes.bfloat16

B, E, R2, T, C, ED, RD = 2048, 20000, 500, 365, 12, 200, 200
NCORES = 8
RS = RD // NCORES       # 25 r's per core
ES = E // NCORES        # 2500 vocab per core
BN_EPS = 1e-5

_cache = {}


def _build(pieces, debug=False):
    """Build + compile the per-core bass kernel. `pieces` is a tuple of
    (coarse_id, col_off, col_len) for branch-2 bucket matmuls (512-grid aligned)."""
    import concourse.bass as bass
    import concourse.mybir as mybir
    import concourse.tile as tile
    from concourse import bacc

    f32 = mybir.dt.float32
    bf16 = mybir.dt.bfloat16

    nc = bacc.Bacc("TRN2", target_bir_lowering=False, debug=False, num_devices=NCORES)

    # ---------------- I/O ----------------
    x1_in = nc.dram_tensor("x1_in", [ED, B], bf16, kind="ExternalInput")   # S1[s_p].T
    x2_in = nc.dram_tensor("x2_in", [ED, B], bf16, kind="ExternalInput")   # S2[s_p].T
    ts_in = nc.dram_tensor("ts_in", [ED, B], bf16, kind="ExternalInput")   # T_S[times_p].T
    to_in = nc.dram_tensor("to_in", [ED, B], bf16, kind="ExternalInput")   # T_O[times_p].T
    g1_in = nc.dram_tensor("g1_in", [100, RS, 2, ED], bf16, kind="ExternalInput")
    g2_in = nc.dram_tensor("g2_in", [RS, 2, 100, C, ED], bf16, kind="ExternalInput")
    pb1_in = nc.dram_tensor("pb1_in", [RS, B], bf16, kind="ExternalInput")  # P1[p_p].T r-slice
    pb2_in = nc.dram_tensor("pb2_in", [RS, B], bf16, kind="ExternalInput")
    # O chunks: feat layout {0:128, 128:200} x {O1, O2}
    oc0_in = nc.dram_tensor("oc0_in", [128, ES], bf16, kind="ExternalInput")
    oc1_in = nc.dram_tensor("oc1_in", [72, ES], bf16, kind="ExternalInput")
    oc2_in = nc.dram_tensor("oc2_in", [128, ES], bf16, kind="ExternalInput")
    oc3_in = nc.dram_tensor("oc3_in", [72, ES], bf16, kind="ExternalInput")
    bnp_in = nc.dram_tensor("bnp_in", [ED, 8], f32, kind="ExternalInput")  # g11,b11,g12,b12,g21,b21,g22,b22
    out_t = nc.dram_tensor("out", [B, ES], f32, kind="ExternalOutput")
    dbg_pe = dbg_x = None
    if debug:
        dbg_pe = nc.dram_tensor("dbg_pe", [2 * ED, B], f32, kind="ExternalOutput")
        dbg_x = nc.dram_tensor("dbg_x", [4, 100, B], bf16, kind="ExternalOutput")

    FS = [(0, 128), (128, 72)]  # feat M-tiles (offset, len)

    with tile.TileContext(nc) as tc:
        from contextlib import ExitStack
        with ExitStack() as ctx:
            singles = ctx.enter_context(tc.tile_pool(name="singles", bufs=1))
            xpool = ctx.enter_context(tc.tile_pool(name="xpool", bufs=1))
            small = ctx.enter_context(tc.tile_pool(name="small", bufs=4))
            btmp = ctx.enter_context(tc.tile_pool(name="btmp", bufs=2))
            perst = ctx.enter_context(tc.tile_pool(name="perst", bufs=1))
            dram = ctx.enter_context(tc.tile_pool(name="dram", bufs=1, space="DRAM"))

            # BN params in both partition alignments
            bnp100 = singles.tile([100, 2, 8], f32)
            nc.sync.dma_start(bnp100[:], bnp_in.rearrange("(h p) c -> p h c", p=100))
            bnpA = singles.tile([128, 8], f32)
            nc.sync.dma_start(bnpA[:], bnp_in[0:128, :])
            bnpB = singles.tile([72, 8], f32)
            nc.sync.dma_start(bnpB[:], bnp_in[128:200, :])
            eps100 = singles.tile([100, 1], f32)
            nc.vector.memset(eps100, BN_EPS)
            eps128 = singles.tile([128, 1], f32)
            nc.vector.memset(eps128, BN_EPS)

            def bn_normalize(src_ap, dst_tile, gcol, bcol, par_ap, eps_tile,
                             postmul=None, premul=None):
                """dst = BN(src [* premul]) * g + b [* postmul] — batch stats along free dim.
                src_ap/dst_tile: [P, B] tiles; gcol/bcol: columns in par_ap [P, ...].
                """
                P = dst_tile.shape[0]
                if premul is not None:
                    pre = btmp.tile([128, B], f32, tag="bn_pre")
                    nc.vector.tensor_tensor(pre[:P], src_ap, premul, mybir.AluOpType.mult)
                    src_ap = pre[:P]
                stats = small.tile([128, 4, 6], f32, tag="bn_stats")
                for i in range(4):
                    nc.vector.bn_stats(stats[:P, i, :], src_ap[:, i * 512:(i + 1) * 512])
                mv = small.tile([128, 2], f32, tag="bn_mv")
                nc.vector.bn_aggr(mv[:P], stats[:P])
                rstd = small.tile([128, 1], f32, tag="bn_rstd")
                nc.scalar.activation(rstd[:P], mv[:P, 1:2], mybir.ActivationFunctionType.Sqrt,
                                     bias=eps_tile[:P], scale=1.0)
                nc.vector.reciprocal(rstd[:P], rstd[:P])
                A = small.tile([128, 1], f32, tag="bn_A")
                nc.vector.tensor_mul(A[:P], rstd[:P], gcol)
                Bt = small.tile([128, 1], f32, tag="bn_B")
                nc.vector.tensor_mul(Bt[:P], mv[:P, 0:1], A[:P])
                nc.vector.tensor_tensor(Bt[:P], bcol, Bt[:P], mybir.AluOpType.subtract)
                nc.vector.tensor_scalar(dst_tile[:], src_ap, A[:P], Bt[:P],
                                        mybir.AluOpType.mult, mybir.AluOpType.add)
                if postmul is not None:
                    nc.vector.tensor_tensor(dst_tile[:], dst_tile[:], postmul,
                                            mybir.AluOpType.mult)

            # ---------- input BN ----------
            # x-side: feat layout in two [100, B] tiles (e-halves)
            x1t = []
            m1t = []
            for h in range(2):
                raw1 = btmp.tile([100, B], bf16, tag="raw_in")
                nc.sync.dma_start(raw1[:], x1_in[100 * h:100 * (h + 1), :])
                d1 = xpool.tile([100, B], bf16, name=f"x1t_{h}")
                bn_normalize(raw1[:], d1, bnp100[:, h, 0:1], bnp100[:, h, 1:2], bnp100, eps100)
                x1t.append(d1)

                raw2 = btmp.tile([100, B], bf16, tag="raw_in")
                nc.sync.dma_start(raw2[:], x2_in[100 * h:100 * (h + 1), :])
                tsh = btmp.tile([100, B], bf16, tag="ts_in")
                nc.sync.dma_start(tsh[:], ts_in[100 * h:100 * (h + 1), :])
                d2 = xpool.tile([100, B], bf16, name=f"m1t_{h}")
                bn_normalize(raw2[:], d2, bnp100[:, h, 4:5], bnp100[:, h, 5:6], bnp100, eps100,
                             postmul=tsh[:])
                m1t.append(d2)

            # ---------- Tucker branches ----------
            pe1_dram = dram.tile([ED, B], f32)
            pe1_out_dram = dram.tile([ED, B], f32, addr_space="Shared")
            pe2_dram = dram.tile([ED, B], f32)
            pe2_out_dram = dram.tile([ED, B], f32, addr_space="Shared")

            with tc.tile_pool(name="tucker", bufs=5) as tpool, \
                 tc.tile_pool(name="gw", bufs=4) as gwpool, \
                 tc.tile_pool(name="psum_tk", bufs=1, space="PSUM") as pst:

                g1_sb = singles.tile([100, RS, 2, ED], bf16)
                for r5 in range(5):
                    nc.sync.dma_start(
                        g1_sb[:, r5 * 5:(r5 + 1) * 5],
                        g1_in[:, r5 * 5:(r5 + 1) * 5])

                # ----- branch 1: full-batch 512 chunks -----
                ps1_a = pst.tile([128, B], f32, tag="ps_m0", name="ps1_a")
                ps1_b = pst.tile([72, B], f32, tag="ps_m1", name="ps1_b")
                ps1 = [ps1_a, ps1_b]
                for r in range(RS):
                    pb = tpool.tile([100, B], bf16, tag="ppb")
                    nc.sync.dma_start(pb[:], pb1_in[r:r + 1, :].partition_broadcast(100).squeeze(1))
                    for h in range(2):
                        z = tpool.tile([100, B], bf16, tag="z")
                        nc.vector.tensor_tensor(z[:], x1t[h][:], pb[:], mybir.AluOpType.mult)
                        first = (r == 0 and h == 0)
                        last = (r == RS - 1 and h == 1)
                        for mi, (mo, ml) in enumerate(FS):
                            for bc in range(4):
                                nc.tensor.matmul(
                                    ps1[mi][:, bc * 512:(bc + 1) * 512],
                                    lhsT=g1_sb[:, r, h, mo:mo + ml],
                                    rhs=z[:, bc * 512:(bc + 1) * 512],
                                    start=first, stop=last)
                # evict branch 1 -> DRAM bounce (rows 0:200)
                for mi, (mo, ml) in enumerate(FS):
                    pe_sb = btmp.tile([128, B], f32, tag="pe_evict")
                    nc.vector.tensor_copy(pe_sb[:ml], ps1[mi][:])
                    nc.sync.dma_start(pe1_dram[mo:mo + ml, :], pe_sb[:ml])
                nc.gpsimd.collective_compute(
                    "AllReduce", mybir.AluOpType.add,
                    replica_groups=[list(range(NCORES))],
                    ins=[pe1_dram.opt()], outs=[pe1_out_dram.opt()])

                # ----- branch 2: per-bucket pieces -----
                ps2_a = pst.tile([128, B], f32, tag="ps_m0", name="ps2_a")
                ps2_b = pst.tile([72, B], f32, tag="ps_m1", name="ps2_b")
                ps2 = [ps2_a, ps2_b]
                for r in range(RS):
                    pb = tpool.tile([100, B], bf16, tag="ppb")
                    nc.sync.dma_start(pb[:], pb2_in[r:r + 1, :].partition_broadcast(100).squeeze(1))
                    for h in range(2):
                        g2c = gwpool.tile([100, C, ED], bf16, tag="g2w")
                        nc.sync.dma_start(g2c[:], g2_in[r, h])
                        z = tpool.tile([100, B], bf16, tag="z")
                        nc.vector.tensor_tensor(z[:], m1t[h][:], pb[:], mybir.AluOpType.mult)
                        first = (r == 0 and h == 0)
                        last = (r == RS - 1 and h == 1)
                        for mi, (mo, ml) in enumerate(FS):
                            seen_banks = set()
                            for (cid, off, ln) in pieces:
                                bank = off // 512
                                bank_first = bank not in seen_banks
                                seen_banks.add(bank)
                                nc.tensor.matmul(
                                    ps2[mi][:, off:off + ln],
                                    lhsT=g2c[:, cid, mo:mo + ml],
                                    rhs=z[:, off:off + ln],
                                    start=(first and bank_first), stop=last,
                                    skip_group_check=True)
                for mi, (mo, ml) in enumerate(FS):
                    pe_sb = btmp.tile([128, B], f32, tag="pe_evict")
                    nc.vector.tensor_copy(pe_sb[:ml], ps2[mi][:])
                    nc.sync.dma_start(pe2_dram[mo:mo + ml, :], pe_sb[:ml])

            # ---------- AllReduce pe2 (pe1's AR already overlaps branch 2) ----------
            nc.gpsimd.collective_compute(
                "AllReduce", mybir.AluOpType.add,
                replica_groups=[list(range(NCORES))],
                ins=[pe2_dram.opt()], outs=[pe2_out_dram.opt()])

            if debug:
                nc.sync.dma_start(dbg_pe[0:ED, :], pe1_out_dram[:, :])
                nc.sync.dma_start(dbg_pe[ED:2 * ED, :], pe2_out_dram[:, :])
                for h in range(2):
                    nc.sync.dma_start(dbg_x[0 + h], x1t[h][:])
                    nc.sync.dma_start(dbg_x[2 + h], m1t[h][:])

            # ---------- post BN + logits ----------
            with tc.tile_pool(name="logits", bufs=4) as lpool, \
                 tc.tile_pool(name="ocat", bufs=1) as opool, \
                 tc.tile_pool(name="psum_l", bufs=4, space="PSUM") as psl:

                oc_sb = []
                for i, (oin, P) in enumerate([(oc0_in, 128), (oc1_in, 72), (oc2_in, 128), (oc3_in, 72)]):
                    t = opool.tile([P, ES], bf16, name=f"oc_{i}")
                    nc.sync.dma_start(t[:], oin[:])
                    oc_sb.append(t)

                # read back reduced pe; branch2 extra T_O multiply; BN with g12/b12, g22/b22
                pe_bn = []
                for br in range(2):
                    for mi, (mo, ml) in enumerate(FS):
                        raw = btmp.tile([128, B], f32, tag="pe_raw")
                        ped = pe1_out_dram if br == 0 else pe2_out_dram
                        nc.sync.dma_start(raw[:ml], ped[mo:mo + ml, :])
                        extra = None
                        if br == 1:
                            toh = btmp.tile([128, B], bf16, tag="to_in")
                            nc.sync.dma_start(toh[:ml], to_in[mo:mo + ml, :])
                            extra = toh[:ml]
                        par = bnpA if mi == 0 else bnpB
                        dst = perst.tile([128, B], bf16, name=f"pebn_{br}_{mi}")
                        gcol = par[:, 2 + 4 * br:3 + 4 * br]
                        bcol = par[:, 3 + 4 * br:4 + 4 * br]
                        bn_normalize(raw[:ml], dst[:ml], gcol, bcol, par, eps128, premul=extra)
                        pe_bn.append((dst, ml))

                # logits matmul + sigmoid + store
                for mb in range(B // 128):
                    orow = lpool.tile([128, ES], f32, tag="orow", bufs=2)
                    for vc in range(ES // 500):
                        psu = psl.tile([128, 512], f32, tag="ps_l")
                        for kc in range(4):
                            peb, kl = pe_bn[kc]
                            nc.tensor.matmul(
                                psu[:, 0:500],
                                lhsT=peb[:kl, mb * 128:(mb + 1) * 128],
                                rhs=oc_sb[kc][:, vc * 500:(vc + 1) * 500],
                                start=(kc == 0), stop=(kc == 3))
                        nc.scalar.activation(orow[:, vc * 500:(vc + 1) * 500], psu[:, 0:500],
                                             mybir.ActivationFunctionType.Sigmoid)
                    nc.sync.dma_start(out_t[mb * 128:(mb + 1) * 128, :], orow[:])

    nc.compile()
    return nc


def kernel(s, p, o, times, fine2coarse, S1, O1, S2, O2, P1, P2, G1, G2, T_S, T_O,
           g11, b11, g12, b12, g21, b21, g22, b22):
    from concourse.bass_utils import run_bass_kernel_spmd

    s = np.asarray(s); p = np.asarray(p); times = np.asarray(times)
    fine2coarse = np.asarray(fine2coarse)

    # ----- host-side routing (index logistics only) -----
    c = fine2coarse[times]                       # [B] coarse id per sample
    perm = np.argsort(c, kind="stable")
    c_sorted = c[perm]
    counts = np.bincount(c_sorted, minlength=C)
    offs = np.concatenate([[0], np.cumsum(counts)])
    pieces = []
    for cid in range(C):
        pos, en = int(offs[cid]), int(offs[cid + 1])
        while pos < en:
            nxt = min(en, (pos // 512 + 1) * 512)
            pieces.append((cid, pos, nxt - pos))
            pos = nxt
    pieces = tuple(pieces)

    key = pieces
    if key not in _cache:
        _cache[key] = _build(pieces)
    nc = _cache[key]

    s_p, p_p, t_p = s[perm], p[perm], times[perm]

    def bt(x):
        return np.ascontiguousarray(x, dtype=BF16)

    x1_in = bt(np.asarray(S1)[s_p].T)
    x2_in = bt(np.asarray(S2)[s_p].T)
    ts_in = bt(np.asarray(T_S)[t_p].T)
    to_in = bt(np.asarray(T_O)[t_p].T)
    pp1 = np.asarray(P1)[p_p]                       # [B, RD]
    pp2 = np.asarray(P2)[p_p]
    G1 = np.asarray(G1); G2 = np.asarray(G2)
    O1 = np.asarray(O1); O2 = np.asarray(O2)
    bnp = np.stack([g11, b11, g12, b12, g21, b21, g22, b22], axis=1).astype(np.float32)
    bnp = np.ascontiguousarray(bnp)

    in_maps = []
    for k in range(NCORES):
        rs = slice(RS * k, RS * (k + 1))
        vs = slice(ES * k, ES * (k + 1))
        g1k = bt(G1[rs].reshape(RS, 2, 100, ED).transpose(2, 0, 1, 3))
        g2k = bt(G2[:, rs].reshape(C, RS, 2, 100, ED).transpose(1, 2, 3, 0, 4))
        pb1 = bt(pp1[:, rs].T)
        pb2 = bt(pp2[:, rs].T)
        o1t = bt(O1[vs].T)   # [200, ES]
        o2t = bt(O2[vs].T)
        in_maps.append({
            "x1_in": x1_in, "x2_in": x2_in, "ts_in": ts_in, "to_in": to_in,
            "g1_in": g1k, "g2_in": g2k, "pb1_in": pb1, "pb2_in": pb2,
            "oc0_in": np.ascontiguousarray(o1t[0:128]),
            "oc1_in": np.ascontiguousarray(o1t[128:200]),
            "oc2_in": np.ascontiguousarray(o2t[0:128]),
            "oc3_in": np.ascontiguousarray(o2t[128:200]),
            "bnp_in": bnp,
        })

    res = run_bass_kernel_spmd(nc, in_maps, core_ids=list(range(NCORES)))

    out_sorted = np.concatenate([res.results[k]["out"] for k in range(NCORES)], axis=1)
    out = np.empty_like(out_sorted)
    out[perm] = out_sorted
    return out


# revision 15
# speedup vs baseline: 31406.0481x; 1.0455x over previous
# kernel.py — self-contained Trainium2 Bass kernel for nn_BTDG_31774168055963 (moe_routing)
#
# Reference computation (see problem):
#   branch1: x1 = BN(S1[s]); pe1 = einsum('be,bef->bf', x1, (P1[p] @ G1.reshape(rd,ed*ed)).reshape(-1,ed,ed))
#            pe1 = BN(pe1); pred1 = pe1 @ O1.T
#   branch2: x2 = BN(S2[s]); m1 = x2*T_S[times]; per-coarse-bucket Tucker core G2[c]
#            pe2 = sum_c [c==fine2coarse[times]] einsum(m1, (P2[p] @ G2[c].reshape(rd,ed*ed)).reshape(-1,ed,ed))
#            pe2 = BN(pe2 * T_O[times]); pred2 = pe2 @ O2.T
#   out = sigmoid(pred1 + pred2)
#
# Strategy (8 NeuronCores):
#   - shard the Tucker rank dim rd=200 -> 25 per core (each core reads 1/8 of G1/G2, perfect balance)
#   - host sorts samples by coarse bucket (pure index routing); kernel does per-bucket matmuls
#   - Tucker contraction via outer-product moving operand Z^T[(r,e),b] = pp[r,b]*x[e,b]
#   - AllReduce the [400, 2048] partial pe across cores, BN on-device, logits matmul sharded
#     column-wise over E=20000 -> 2500 per core, sigmoid on ScalarE, host concatenates+unpermutes.
#   - all matmuls bf16 (fp32 PSUM accumulation); BN statistics fp32.

import numpy as np
import ml_dtypes

BF16 = ml_dtypes.bfloat16

B, E, R2, T, C, ED, RD = 2048, 20000, 500, 365, 12, 200, 200
NCORES = 8
RS = RD // NCORES       # 25 r's per core
ES = E // NCORES        # 2500 vocab per core
BN_EPS = 1e-5

_cache = {}


def _build(pieces, debug=False):
    """Build + compile the per-core bass kernel. `pieces` is a tuple of
    (coarse_id, col_off, col_len) for branch-2 bucket matmuls (512-grid aligned)."""
    import concourse.bass as bass
    import concourse.mybir as mybir
    import concourse.tile as tile
    from concourse import bacc

    f32 = mybir.dt.float32
    bf16 = mybir.dt.bfloat16

    nc = bacc.Bacc("TRN2", target_bir_lowering=False, debug=False, num_devices=NCORES)

    # ---------------- I/O ----------------
    x1_in = nc.dram_tensor("x1_in", [ED, B], bf16, kind="ExternalInput")   # S1[s_p].T
    x2_in = nc.dram_tensor("x2_in", [ED, B], bf16, kind="ExternalInput")   # S2[s_p].T
    ts_in = nc.dram_tensor("ts_in", [ED, B], bf16, kind="ExternalInput")   # T_S[times_p].T
    to_in = nc.dram_tensor("to_in", [ED, B], bf16, kind="ExternalInput")   # T_O[times_p].T
    g1_in = nc.dram_tensor("g1_in", [100, RS, 2, ED], bf16, kind="ExternalInput")
    g2_in = nc.dram_tensor("g2_in", [RS, 2, 100, C, ED], bf16, kind="ExternalInput")
    pb1_in = nc.dram_tensor("pb1_in", [RS, B], bf16, kind="ExternalInput")  # P1[p_p].T r-slice
    pb2_in = nc.dram_tensor("pb2_in", [RS, B], bf16, kind="ExternalInput")
    # O chunks: feat layout {0:128, 128:200} x {O1, O2}
    oc0_in = nc.dram_tensor("oc0_in", [128, ES], bf16, kind="ExternalInput")
    oc1_in = nc.dram_tensor("oc1_in", [72, ES], bf16, kind="ExternalInput")
    oc2_in = nc.dram_tensor("oc2_in", [128, ES], bf16, kind="ExternalInput")
    oc3_in = nc.dram_tensor("oc3_in", [72, ES], bf16, kind="ExternalInput")
    bnp_in = nc.dram_tensor("bnp_in", [ED, 8], f32, kind="ExternalInput")  # g11,b11,g12,b12,g21,b21,g22,b22
    out_t = nc.dram_tensor("out", [B, ES], f32, kind="ExternalOutput")
    dbg_pe = dbg_x = None
    if debug:
        dbg_pe = nc.dram_tensor("dbg_pe", [2 * ED, B], f32, kind="ExternalOutput")
        dbg_x = nc.dram_tensor("dbg_x", [4, 100, B], bf16, kind="ExternalOutput")

    FS = [(0, 128), (128, 72)]  # feat M-tiles (offset, len)

    with tile.TileContext(nc) as tc:
        from contextlib import ExitStack
        with ExitStack() as ctx:
            singles = ctx.enter_context(tc.tile_pool(name="singles", bufs=1))
            xpool = ctx.enter_context(tc.tile_pool(name="xpool", bufs=1))
            small = ctx.enter_context(tc.tile_pool(name="small", bufs=4))
            btmp = ctx.enter_context(tc.tile_pool(name="btmp", bufs=2))
            perst = ctx.enter_context(tc.tile_pool(name="perst", bufs=1))
            dram = ctx.enter_context(tc.tile_pool(name="dram", bufs=1, space="DRAM"))

            # BN params in both partition alignments
            bnp100 = singles.tile([100, 2, 8], f32)
            nc.sync.dma_start(bnp100[:], bnp_in.rearrange("(h p) c -> p h c", p=100))
            bnpA = singles.tile([128, 8], f32)
            nc.sync.dma_start(bnpA[:], bnp_in[0:128, :])
            bnpB = singles.tile([72, 8], f32)
            nc.sync.dma_start(bnpB[:], bnp_in[128:200, :])
            eps100 = singles.tile([100, 1], f32)
            nc.vector.memset(eps100, BN_EPS)
            eps128 = singles.tile([128, 1], f32)
            nc.vector.memset(eps128, BN_EPS)

            def bn_normalize(src_ap, dst_tile, gcol, bcol, par_ap, eps_tile,
                             postmul=None, premul=None):
                """dst = BN(src [* premul]) * g + b [* postmul] — batch stats along free dim.
                src_ap/dst_tile: [P, B] tiles; gcol/bcol: columns in par_ap [P, ...].
                """
                P = dst_tile.shape[0]
                if premul is not None:
                    pre = btmp.tile([128, B], f32, tag="bn_pre")
                    nc.vector.tensor_tensor(pre[:P], src_ap, premul, mybir.AluOpType.mult)
                    src_ap = pre[:P]
                stats = small.tile([128, 4, 6], f32, tag="bn_stats")
                for i in range(4):
                    nc.vector.bn_stats(stats[:P, i, :], src_ap[:, i * 512:(i + 1) * 512])
                mv = small.tile([128, 2], f32, tag="bn_mv")
                nc.vector.bn_aggr(mv[:P], stats[:P])
                rstd = small.tile([128, 1], f32, tag="bn_rstd")
                nc.scalar.activation(rstd[:P], mv[:P, 1:2], mybir.ActivationFunctionType.Sqrt,
                                     bias=eps_tile[:P], scale=1.0)
                nc.vector.reciprocal(rstd[:P], rstd[:P])
                A = small.tile([128, 1], f32, tag="bn_A")
                nc.vector.tensor_mul(A[:P], rstd[:P], gcol)
                Bt = small.tile([128, 1], f32, tag="bn_B")
                nc.vector.tensor_mul(Bt[:P], mv[:P, 0:1], A[:P])
                nc.vector.tensor_tensor(Bt[:P], bcol, Bt[:P], mybir.AluOpType.subtract)
                nc.vector.tensor_scalar(dst_tile[:], src_ap, A[:P], Bt[:P],
                                        mybir.AluOpType.mult, mybir.AluOpType.add)
                if postmul is not None:
                    nc.vector.tensor_tensor(dst_tile[:], dst_tile[:], postmul,
                                            mybir.AluOpType.mult)

            # ---------- input BN ----------
            # x-side: feat layout in two [100, B] tiles (e-halves)
            x1t = []
            m1t = []
            for h in range(2):
                raw1 = btmp.tile([100, B], bf16, tag="raw_in")
                nc.sync.dma_start(raw1[:], x1_in[100 * h:100 * (h + 1), :])
                d1 = xpool.tile([100, B], bf16, name=f"x1t_{h}")
                bn_normalize(raw1[:], d1, bnp100[:, h, 0:1], bnp100[:, h, 1:2], bnp100, eps100)
                x1t.append(d1)
            for h in range(2):
                raw2 = btmp.tile([100, B], bf16, tag="raw_in")
                nc.sync.dma_start(raw2[:], x2_in[100 * h:100 * (h + 1), :])
                tsh = btmp.tile([100, B], bf16, tag="ts_in")
                nc.sync.dma_start(tsh[:], ts_in[100 * h:100 * (h + 1), :])
                d2 = xpool.tile([100, B], bf16, name=f"m1t_{h}")
                bn_normalize(raw2[:], d2, bnp100[:, h, 4:5], bnp100[:, h, 5:6], bnp100, eps100,
                             postmul=tsh[:])
                m1t.append(d2)

            # ---------- Tucker branches ----------
            pe1_dram = dram.tile([ED, B], f32)
            pe1_out_dram = dram.tile([ED, B], f32, addr_space="Shared")
            pe2_dram = dram.tile([ED, B], bf16)
            pe2_out_dram = dram.tile([ED, B], bf16, addr_space="Shared")

            with tc.tile_pool(name="tucker", bufs=5) as tpool, \
                 tc.tile_pool(name="gw", bufs=4) as gwpool, \
                 tc.tile_pool(name="psum_tk", bufs=1, space="PSUM") as pst:

                g1_sb = singles.tile([100, RS, 2, ED], bf16)
                for r5 in range(5):
                    nc.sync.dma_start(
                        g1_sb[:, r5 * 5:(r5 + 1) * 5],
                        g1_in[:, r5 * 5:(r5 + 1) * 5])

                # ----- branch 1: full-batch 512 chunks -----
                ps1_a = pst.tile([128, B], f32, tag="ps_m0", name="ps1_a")
                ps1_b = pst.tile([72, B], f32, tag="ps_m1", name="ps1_b")
                ps1 = [ps1_a, ps1_b]
                for r in range(RS):
                    pb = tpool.tile([100, B], bf16, tag="ppb")
                    nc.sync.dma_start(pb[:], pb1_in[r:r + 1, :].partition_broadcast(100).squeeze(1))
                    for h in range(2):
                        z = tpool.tile([100, B], bf16, tag="z")
                        nc.vector.tensor_tensor(z[:], x1t[h][:], pb[:], mybir.AluOpType.mult)
                        first = (r == 0 and h == 0)
                        last = (r == RS - 1 and h == 1)
                        for mi, (mo, ml) in enumerate(FS):
                            for bc in range(4):
                                nc.tensor.matmul(
                                    ps1[mi][:, bc * 512:(bc + 1) * 512],
                                    lhsT=g1_sb[:, r, h, mo:mo + ml],
                                    rhs=z[:, bc * 512:(bc + 1) * 512],
                                    start=first, stop=last)
                # evict branch 1 -> DRAM bounce (rows 0:200)
                for mi, (mo, ml) in enumerate(FS):
                    pe_sb = btmp.tile([128, B], f32, tag="pe_evict")
                    nc.vector.tensor_copy(pe_sb[:ml], ps1[mi][:])
                    nc.sync.dma_start(pe1_dram[mo:mo + ml, :], pe_sb[:ml])
                nc.gpsimd.collective_compute(
                    "AllReduce", mybir.AluOpType.add,
                    replica_groups=[list(range(NCORES))],
                    ins=[pe1_dram.opt()], outs=[pe1_out_dram.opt()])

                # ----- branch 2: per-bucket pieces -----
                ps2_a = pst.tile([128, B], f32, tag="ps_m0", name="ps2_a")
                ps2_b = pst.tile([72, B], f32, tag="ps_m1", name="ps2_b")
                ps2 = [ps2_a, ps2_b]
                for r in range(RS):
                    pb = tpool.tile([100, B], bf16, tag="ppb")
                    nc.sync.dma_start(pb[:], pb2_in[r:r + 1, :].partition_broadcast(100).squeeze(1))
                    for h in range(2):
                        g2c = gwpool.tile([100, C, ED], bf16, tag="g2w")
                        nc.sync.dma_start(g2c[:], g2_in[r, h])
                        z = tpool.tile([100, B], bf16, tag="z")
                        nc.vector.tensor_tensor(z[:], m1t[h][:], pb[:], mybir.AluOpType.mult)
                        first = (r == 0 and h == 0)
                        last = (r == RS - 1 and h == 1)
                        for mi, (mo, ml) in enumerate(FS):
                            seen_banks = set()
                            for (cid, off, ln) in pieces:
                                bank = off // 512
                                bank_first = bank not in seen_banks
                                seen_banks.add(bank)
                                nc.tensor.matmul(
                                    ps2[mi][:, off:off + ln],
                                    lhsT=g2c[:, cid, mo:mo + ml],
                                    rhs=z[:, off:off + ln],
                                    start=(first and bank_first), stop=last,
                                    skip_group_check=True)
                for mi, (mo, ml) in enumerate(FS):
                    pe_sb2 = btmp.tile([128, B], bf16, tag="pe_evict2")
                    nc.vector.tensor_copy(pe_sb2[:ml], ps2[mi][:])
                    nc.sync.dma_start(pe2_dram[mo:mo + ml, :], pe_sb2[:ml])

            # ---------- AllReduce pe2 (pe1's AR already overlaps branch 2) ----------
            nc.gpsimd.collective_compute(
                "AllReduce", mybir.AluOpType.add,
                replica_groups=[list(range(NCORES))],
                ins=[pe2_dram.opt()], outs=[pe2_out_dram.opt()])

            if debug:
                nc.sync.dma_start(dbg_pe[0:ED, :], pe1_out_dram[:, :])
                for h in range(2):
                    nc.sync.dma_start(dbg_x[0 + h], x1t[h][:])
                    nc.sync.dma_start(dbg_x[2 + h], m1t[h][:])

            # ---------- post BN + logits ----------
            with tc.tile_pool(name="logits", bufs=4) as lpool, \
                 tc.tile_pool(name="ocat", bufs=1) as opool, \
                 tc.tile_pool(name="psum_l", bufs=6, space="PSUM") as psl:

                oc_sb = []
                for i, (oin, P) in enumerate([(oc0_in, 128), (oc1_in, 72), (oc2_in, 128), (oc3_in, 72)]):
                    t = opool.tile([P, ES], bf16, name=f"oc_{i}")
                    nc.sync.dma_start(t[:], oin[:])
                    oc_sb.append(t)

                # read back reduced pe; branch2 extra T_O multiply; BN with g12/b12, g22/b22
                pe_bn = []
                for br in range(2):
                    for mi, (mo, ml) in enumerate(FS):
                        rdt = f32 if br == 0 else bf16
                        raw = btmp.tile([128, B], rdt, tag=f"pe_raw{br}")
                        ped = pe1_out_dram if br == 0 else pe2_out_dram
                        nc.sync.dma_start(raw[:ml], ped[mo:mo + ml, :])
                        extra = None
                        if br == 1:
                            toh = btmp.tile([128, B], bf16, tag="to_in")
                            nc.sync.dma_start(toh[:ml], to_in[mo:mo + ml, :])
                            extra = toh[:ml]
                        par = bnpA if mi == 0 else bnpB
                        dst = perst.tile([128, B], bf16, name=f"pebn_{br}_{mi}")
                        gcol = par[:, 2 + 4 * br:3 + 4 * br]
                        bcol = par[:, 3 + 4 * br:4 + 4 * br]
                        bn_normalize(raw[:ml], dst[:ml], gcol, bcol, par, eps128, premul=extra)
                        pe_bn.append((dst, ml))

                # logits matmul + sigmoid + store
                for mb in range(B // 128):
                    orow = lpool.tile([128, ES], f32, tag="orow", bufs=2)
                    for vc in range(ES // 500):
                        psu = psl.tile([128, 512], f32, tag="ps_l")
                        for kc in range(4):
                            peb, kl = pe_bn[kc]
                            nc.tensor.matmul(
                                psu[:, 0:500],
                                lhsT=peb[:kl, mb * 128:(mb + 1) * 128],
                                rhs=oc_sb[kc][:, vc * 500:(vc + 1) * 500],
                                start=(kc == 0), stop=(kc == 3))
                        nc.scalar.activation(orow[:, vc * 500:(vc + 1) * 500], psu[:, 0:500],
                                             mybir.ActivationFunctionType.Sigmoid)
                    nc.sync.dma_start(out_t[mb * 128:(mb + 1) * 128, :], orow[:])

    nc.compile()
    return nc


def kernel(s, p, o, times, fine2coarse, S1, O1, S2, O2, P1, P2, G1, G2, T_S, T_O,
           g11, b11, g12, b12, g21, b21, g22, b22):
    from concourse.bass_utils import run_bass_kernel_spmd

    s = np.asarray(s); p = np.asarray(p); times = np.asarray(times)
    fine2coarse = np.asarray(fine2coarse)

    # ----- host-side routing (index logistics only) -----
    c = fine2coarse[times]                       # [B] coarse id per sample
    perm = np.argsort(c, kind="stable")
    c_sorted = c[perm]
    counts = np.bincount(c_sorted, minlength=C)
    offs = np.concatenate([[0], np.cumsum(counts)])
    pieces = []
    for cid in range(C):
        pos, en = int(offs[cid]), int(offs[cid + 1])
        while pos < en:
            nxt = min(en, (pos // 512 + 1) * 512)
            pieces.append((cid, pos, nxt - pos))
            pos = nxt
    pieces = tuple(pieces)

    key = pieces
    if key not in _cache:
        _cache[key] = _build(pieces)
    nc = _cache[key]

    s_p, p_p, t_p = s[perm], p[perm], times[perm]

    def bt(x):
        return np.ascontiguousarray(x, dtype=BF16)

    x1_in = bt(np.asarray(S1)[s_p].T)
    x2_in = bt(np.asarray(S2)[s_p].T)
    ts_in = bt(np.asarray(T_S)[t_p].T)
    to_in = bt(np.asarray(T_O)[t_p].T)
    pp1 = np.asarray(P1)[p_p]                       # [B, RD]
    pp2 = np.asarray(P2)[p_p]
    G1 = np.asarray(G1); G2 = np.asarray(G2)
    O1 = np.asarray(O1); O2 = np.asarray(O2)
    bnp = np.stack([g11, b11, g12, b12, g21, b21, g22, b22], axis=1).astype(np.float32)
    bnp = np.ascontiguousarray(bnp)

    in_maps = []
    for k in range(NCORES):
        rs = slice(RS * k, RS * (k + 1))
        vs = slice(ES * k, ES * (k + 1))
        g1k = bt(G1[rs].reshape(RS, 2, 100, ED).transpose(2, 0, 1, 3))
        g2k = bt(G2[:, rs].reshape(C, RS, 2, 100, ED).transpose(1, 2, 3, 0, 4))
        pb1 = bt(pp1[:, rs].T)
        pb2 = bt(pp2[:, rs].T)
        o1t = bt(O1[vs].T)   # [200, ES]
        o2t = bt(O2[vs].T)
        in_maps.append({
            "x1_in": x1_in, "x2_in": x2_in, "ts_in": ts_in, "to_in": to_in,
            "g1_in": g1k, "g2_in": g2k, "pb1_in": pb1, "pb2_in": pb2,
            "oc0_in": np.ascontiguousarray(o1t[0:128]),
            "oc1_in": np.ascontiguousarray(o1t[128:200]),
            "oc2_in": np.ascontiguousarray(o2t[0:128]),
            "oc3_in": np.ascontiguousarray(o2t[128:200]),
            "bnp_in": bnp,
        })

    res = run_bass_kernel_spmd(nc, in_maps, core_ids=list(range(NCORES)))

    out_sorted = np.concatenate([res.results[k]["out"] for k in range(NCORES)], axis=1)
    out = np.empty_like(out_sorted)
    out[perm] = out_sorted
    return out


# revision 18
# speedup vs baseline: 31820.8401x; 1.0132x over previous
# kernel.py — self-contained Trainium2 Bass kernel for nn_BTDG_31774168055963 (moe_routing)
#
# Reference computation (see problem):
#   branch1: x1 = BN(S1[s]); pe1 = einsum('be,bef->bf', x1, (P1[p] @ G1.reshape(rd,ed*ed)).reshape(-1,ed,ed))
#            pe1 = BN(pe1); pred1 = pe1 @ O1.T
#   branch2: x2 = BN(S2[s]); m1 = x2*T_S[times]; per-coarse-bucket Tucker core G2[c]
#            pe2 = sum_c [c==fine2coarse[times]] einsum(m1, (P2[p] @ G2[c].reshape(rd,ed*ed)).reshape(-1,ed,ed))
#            pe2 = BN(pe2 * T_O[times]); pred2 = pe2 @ O2.T
#   out = sigmoid(pred1 + pred2)
#
# Strategy (8 NeuronCores):
#   - shard the Tucker rank dim rd=200 -> 25 per core (each core reads 1/8 of G1/G2, perfect balance)
#   - host sorts samples by coarse bucket (pure index routing); kernel does per-bucket matmuls
#   - Tucker contraction via outer-product moving operand Z^T[(r,e),b] = pp[r,b]*x[e,b]
#   - AllReduce the [400, 2048] partial pe across cores, BN on-device, logits matmul sharded
#     column-wise over E=20000 -> 2500 per core, sigmoid on ScalarE, host concatenates+unpermutes.
#   - all matmuls bf16 (fp32 PSUM accumulation); BN statistics fp32.

import numpy as np
import ml_dtypes

BF16 = ml_dtypes.bfloat16

B, E, R2, T, C, ED, RD = 2048, 20000, 500, 365, 12, 200, 200
NCORES = 8
RS = RD // NCORES       # 25 r's per core
ES = E // NCORES        # 2500 vocab per core
BN_EPS = 1e-5

_cache = {}


def _build(pieces, debug=False):
    """Build + compile the per-core bass kernel. `pieces` is a tuple of
    (coarse_id, col_off, col_len) for branch-2 bucket matmuls (512-grid aligned)."""
    import concourse.bass as bass
    import concourse.mybir as mybir
    import concourse.tile as tile
    from concourse import bacc

    f32 = mybir.dt.float32
    bf16 = mybir.dt.bfloat16

    nc = bacc.Bacc("TRN2", target_bir_lowering=False, debug=False, num_devices=NCORES)

    # ---------------- I/O ----------------
    x1_in = nc.dram_tensor("x1_in", [ED, B], bf16, kind="ExternalInput")   # S1[s_p].T
    x2_in = nc.dram_tensor("x2_in", [ED, B], bf16, kind="ExternalInput")   # S2[s_p].T
    ts_in = nc.dram_tensor("ts_in", [ED, B], bf16, kind="ExternalInput")   # T_S[times_p].T
    to_in = nc.dram_tensor("to_in", [ED, B], bf16, kind="ExternalInput")   # T_O[times_p].T
    g1_in = nc.dram_tensor("g1_in", [100, RS, 2, ED], bf16, kind="ExternalInput")
    g2_in = nc.dram_tensor("g2_in", [RS, 2, 100, C, ED], bf16, kind="ExternalInput")
    pb1_in = nc.dram_tensor("pb1_in", [RS, B], bf16, kind="ExternalInput")  # P1[p_p].T r-slice
    pb2_in = nc.dram_tensor("pb2_in", [RS, B], bf16, kind="ExternalInput")
    # O chunks: feat layout {0:128, 128:200} x {O1, O2}
    oc0_in = nc.dram_tensor("oc0_in", [128, ES], bf16, kind="ExternalInput")
    oc1_in = nc.dram_tensor("oc1_in", [72, ES], bf16, kind="ExternalInput")
    oc2_in = nc.dram_tensor("oc2_in", [128, ES], bf16, kind="ExternalInput")
    oc3_in = nc.dram_tensor("oc3_in", [72, ES], bf16, kind="ExternalInput")
    bnp_in = nc.dram_tensor("bnp_in", [ED, 8], f32, kind="ExternalInput")  # g11,b11,g12,b12,g21,b21,g22,b22
    out_t = nc.dram_tensor("out", [B, ES], f32, kind="ExternalOutput")
    dbg_pe = dbg_x = None
    if debug:
        dbg_pe = nc.dram_tensor("dbg_pe", [2 * ED, B], f32, kind="ExternalOutput")
        dbg_x = nc.dram_tensor("dbg_x", [4, 100, B], bf16, kind="ExternalOutput")

    FS = [(0, 128), (128, 72)]  # feat M-tiles (offset, len)

    with tile.TileContext(nc) as tc:
        from contextlib import ExitStack
        with ExitStack() as ctx:
            singles = ctx.enter_context(tc.tile_pool(name="singles", bufs=1))
            xpool = ctx.enter_context(tc.tile_pool(name="xpool", bufs=1))
            small = ctx.enter_context(tc.tile_pool(name="small", bufs=4))
            btmp = ctx.enter_context(tc.tile_pool(name="btmp", bufs=2))
            perst = ctx.enter_context(tc.tile_pool(name="perst", bufs=1))
            dram = ctx.enter_context(tc.tile_pool(name="dram", bufs=1, space="DRAM"))

            # BN params in both partition alignments
            bnp100 = singles.tile([100, 2, 8], f32)
            nc.sync.dma_start(bnp100[:], bnp_in.rearrange("(h p) c -> p h c", p=100))
            bnpA = singles.tile([128, 8], f32)
            nc.sync.dma_start(bnpA[:], bnp_in[0:128, :])
            bnpB = singles.tile([72, 8], f32)
            nc.sync.dma_start(bnpB[:], bnp_in[128:200, :])
            eps100 = singles.tile([100, 1], f32)
            nc.vector.memset(eps100, BN_EPS)
            eps128 = singles.tile([128, 1], f32)
            nc.vector.memset(eps128, BN_EPS)

            def bn_normalize(src_ap, dst_tile, gcol, bcol, par_ap, eps_tile,
                             postmul=None, premul=None):
                """dst = BN(src [* premul]) * g + b [* postmul] — batch stats along free dim.
                src_ap/dst_tile: [P, B] tiles; gcol/bcol: columns in par_ap [P, ...].
                """
                P = dst_tile.shape[0]
                if premul is not None:
                    pre = btmp.tile([128, B], f32, tag="bn_pre")
                    nc.vector.tensor_tensor(pre[:P], src_ap, premul, mybir.AluOpType.mult)
                    src_ap = pre[:P]
                stats = small.tile([128, 4, 6], f32, tag="bn_stats")
                for i in range(4):
                    nc.vector.bn_stats(stats[:P, i, :], src_ap[:, i * 512:(i + 1) * 512])
                mv = small.tile([128, 2], f32, tag="bn_mv")
                nc.vector.bn_aggr(mv[:P], stats[:P])
                rstd = small.tile([128, 1], f32, tag="bn_rstd")
                nc.scalar.activation(rstd[:P], mv[:P, 1:2], mybir.ActivationFunctionType.Sqrt,
                                     bias=eps_tile[:P], scale=1.0)
                nc.vector.reciprocal(rstd[:P], rstd[:P])
                A = small.tile([128, 1], f32, tag="bn_A")
                nc.vector.tensor_mul(A[:P], rstd[:P], gcol)
                Bt = small.tile([128, 1], f32, tag="bn_B")
                nc.vector.tensor_mul(Bt[:P], mv[:P, 0:1], A[:P])
                nc.vector.tensor_tensor(Bt[:P], bcol, Bt[:P], mybir.AluOpType.subtract)
                nc.vector.tensor_scalar(dst_tile[:], src_ap, A[:P], Bt[:P],
                                        mybir.AluOpType.mult, mybir.AluOpType.add)
                if postmul is not None:
                    nc.vector.tensor_tensor(dst_tile[:], dst_tile[:], postmul,
                                            mybir.AluOpType.mult)

            # ---------- input BN ----------
            # x-side: feat layout in two [100, B] tiles (e-halves)
            x1t = []
            m1t = []
            for h in range(2):
                raw1 = btmp.tile([100, B], bf16, tag="raw_in")
                nc.sync.dma_start(raw1[:], x1_in[100 * h:100 * (h + 1), :])
                d1 = xpool.tile([100, B], bf16, name=f"x1t_{h}")
                bn_normalize(raw1[:], d1, bnp100[:, h, 0:1], bnp100[:, h, 1:2], bnp100, eps100)
                x1t.append(d1)

            # ---------- Tucker branches ----------
            pe1_dram = dram.tile([ED, B], f32)
            pe1_out_dram = dram.tile([ED, B], f32, addr_space="Shared")
            pe2_dram = dram.tile([ED, B], bf16)
            pe2_out_dram = dram.tile([ED, B], bf16, addr_space="Shared")

            with tc.tile_pool(name="tucker", bufs=5) as tpool, \
                 tc.tile_pool(name="gw", bufs=5) as gwpool, \
                 tc.tile_pool(name="psum_tk", bufs=1, space="PSUM") as pst:

                g1_sb = singles.tile([100, RS, 2, ED], bf16)
                for r5 in range(5):
                    nc.sync.dma_start(
                        g1_sb[:, r5 * 5:(r5 + 1) * 5],
                        g1_in[:, r5 * 5:(r5 + 1) * 5])

                # ----- branch 1: full-batch 512 chunks -----
                ps1_a = pst.tile([128, B], f32, tag="ps_m0", name="ps1_a")
                ps1_b = pst.tile([72, B], f32, tag="ps_m1", name="ps1_b")
                ps1 = [ps1_a, ps1_b]
                for r in range(RS):
                    pb = tpool.tile([100, B], bf16, tag="ppb")
                    nc.sync.dma_start(pb[:], pb1_in[r:r + 1, :].partition_broadcast(100).squeeze(1))
                    for h in range(2):
                        z = tpool.tile([100, B], bf16, tag="z")
                        nc.vector.tensor_tensor(z[:], x1t[h][:], pb[:], mybir.AluOpType.mult)
                        first = (r == 0 and h == 0)
                        last = (r == RS - 1 and h == 1)
                        for mi, (mo, ml) in enumerate(FS):
                            for bc in range(4):
                                nc.tensor.matmul(
                                    ps1[mi][:, bc * 512:(bc + 1) * 512],
                                    lhsT=g1_sb[:, r, h, mo:mo + ml],
                                    rhs=z[:, bc * 512:(bc + 1) * 512],
                                    start=first, stop=last)
                # evict branch 1 -> DRAM bounce (rows 0:200)
                for mi, (mo, ml) in enumerate(FS):
                    pe_sb = btmp.tile([128, B], f32, tag="pe_evict")
                    nc.vector.tensor_copy(pe_sb[:ml], ps1[mi][:])
                    nc.sync.dma_start(pe1_dram[mo:mo + ml, :], pe_sb[:ml])
                nc.gpsimd.collective_compute(
                    "AllReduce", mybir.AluOpType.add,
                    replica_groups=[list(range(NCORES))],
                    ins=[pe1_dram.opt()], outs=[pe1_out_dram.opt()])

                # x2/m1 BN (overlaps branch-1 Tucker)
                for h in range(2):
                    raw2 = btmp.tile([100, B], bf16, tag="raw_in")
                    nc.sync.dma_start(raw2[:], x2_in[100 * h:100 * (h + 1), :])
                    tsh = btmp.tile([100, B], bf16, tag="ts_in")
                    nc.sync.dma_start(tsh[:], ts_in[100 * h:100 * (h + 1), :])
                    d2 = xpool.tile([100, B], bf16, name=f"m1t_{h}")
                    bn_normalize(raw2[:], d2, bnp100[:, h, 4:5], bnp100[:, h, 5:6], bnp100, eps100,
                                 postmul=tsh[:])
                    m1t.append(d2)

                # ----- branch 2: per-bucket pieces -----
                ps2_a = pst.tile([128, B], f32, tag="ps_m0", name="ps2_a")
                ps2_b = pst.tile([72, B], f32, tag="ps_m1", name="ps2_b")
                ps2 = [ps2_a, ps2_b]
                for r in range(RS):
                    pb = tpool.tile([100, B], bf16, tag="ppb")
                    nc.sync.dma_start(pb[:], pb2_in[r:r + 1, :].partition_broadcast(100).squeeze(1))
                    for h in range(2):
                        g2c = gwpool.tile([100, C, ED], bf16, tag="g2w")
                        nc.sync.dma_start(g2c[:], g2_in[r, h])
                        z = tpool.tile([100, B], bf16, tag="z")
                        nc.vector.tensor_tensor(z[:], m1t[h][:], pb[:], mybir.AluOpType.mult)
                        first = (r == 0 and h == 0)
                        last = (r == RS - 1 and h == 1)
                        for mi, (mo, ml) in enumerate(FS):
                            seen_banks = set()
                            for (cid, off, ln) in pieces:
                                bank = off // 512
                                bank_first = bank not in seen_banks
                                seen_banks.add(bank)
                                nc.tensor.matmul(
                                    ps2[mi][:, off:off + ln],
                                    lhsT=g2c[:, cid, mo:mo + ml],
                                    rhs=z[:, off:off + ln],
                                    start=(first and bank_first), stop=last,
                                    skip_group_check=True)
                for mi, (mo, ml) in enumerate(FS):
                    pe_sb2 = btmp.tile([128, B], bf16, tag="pe_evict2")
                    nc.vector.tensor_copy(pe_sb2[:ml], ps2[mi][:])
                    nc.sync.dma_start(pe2_dram[mo:mo + ml, :], pe_sb2[:ml])

            # ---------- AllReduce pe2 (pe1's AR already overlaps branch 2) ----------
            nc.gpsimd.collective_compute(
                "AllReduce", mybir.AluOpType.add,
                replica_groups=[list(range(NCORES))],
                ins=[pe2_dram.opt()], outs=[pe2_out_dram.opt()])

            if debug:
                nc.sync.dma_start(dbg_pe[0:ED, :], pe1_out_dram[:, :])
                for h in range(2):
                    nc.sync.dma_start(dbg_x[0 + h], x1t[h][:])
                    nc.sync.dma_start(dbg_x[2 + h], m1t[h][:])

            # ---------- post BN + logits ----------
            with tc.tile_pool(name="logits", bufs=4) as lpool, \
                 tc.tile_pool(name="ocat", bufs=1) as opool, \
                 tc.tile_pool(name="psum_l", bufs=6, space="PSUM") as psl:

                oc_sb = []
                for i, (oin, P) in enumerate([(oc0_in, 128), (oc1_in, 72), (oc2_in, 128), (oc3_in, 72)]):
                    t = opool.tile([P, ES], bf16, name=f"oc_{i}")
                    nc.sync.dma_start(t[:], oin[:])
                    oc_sb.append(t)

                # read back reduced pe; branch2 extra T_O multiply; BN with g12/b12, g22/b22
                pe_bn = []
                for br in range(2):
                    for mi, (mo, ml) in enumerate(FS):
                        rdt = f32 if br == 0 else bf16
                        raw = btmp.tile([128, B], rdt, tag=f"pe_raw{br}")
                        ped = pe1_out_dram if br == 0 else pe2_out_dram
                        nc.sync.dma_start(raw[:ml], ped[mo:mo + ml, :])
                        extra = None
                        if br == 1:
                            toh = btmp.tile([128, B], bf16, tag="to_in")
                            nc.sync.dma_start(toh[:ml], to_in[mo:mo + ml, :])
                            extra = toh[:ml]
                        par = bnpA if mi == 0 else bnpB
                        dst = perst.tile([128, B], bf16, name=f"pebn_{br}_{mi}")
                        gcol = par[:, 2 + 4 * br:3 + 4 * br]
                        bcol = par[:, 3 + 4 * br:4 + 4 * br]
                        bn_normalize(raw[:ml], dst[:ml], gcol, bcol, par, eps128, premul=extra)
                        pe_bn.append((dst, ml))

                # logits matmul + sigmoid + store
                for mb in range(B // 128):
                    orow = lpool.tile([128, ES], f32, tag="orow", bufs=2)
                    for vc in range(ES // 500):
                        psu = psl.tile([128, 512], f32, tag="ps_l")
                        for kc in range(4):
                            peb, kl = pe_bn[kc]
                            nc.tensor.matmul(
                                psu[:, 0:500],
                                lhsT=peb[:kl, mb * 128:(mb + 1) * 128],
                                rhs=oc_sb[kc][:, vc * 500:(vc + 1) * 500],
                                start=(kc == 0), stop=(kc == 3))
                        nc.scalar.activation(orow[:, vc * 500:(vc + 1) * 500], psu[:, 0:500],
                                             mybir.ActivationFunctionType.Sigmoid)
                    nc.sync.dma_start(out_t[mb * 128:(mb + 1) * 128, :], orow[:])

    nc.compile()
    return nc


def kernel(s, p, o, times, fine2coarse, S1, O1, S2, O2, P1, P2, G1, G2, T_S, T_O,
           g11, b11, g12, b12, g21, b21, g22, b22):
    from concourse.bass_utils import run_bass_kernel_spmd

    s = np.asarray(s); p = np.asarray(p); times = np.asarray(times)
    fine2coarse = np.asarray(fine2coarse)

    # ----- host-side routing (index logistics only) -----
    c = fine2coarse[times]                       # [B] coarse id per sample
    perm = np.argsort(c, kind="stable")
    c_sorted = c[perm]
    counts = np.bincount(c_sorted, minlength=C)
    offs = np.concatenate([[0], np.cumsum(counts)])
    pieces = []
    for cid in range(C):
        pos, en = int(offs[cid]), int(offs[cid + 1])
        while pos < en:
            nxt = min(en, (pos // 512 + 1) * 512)
            pieces.append((cid, pos, nxt - pos))
            pos = nxt
    pieces = tuple(pieces)

    key = pieces
    if key not in _cache:
        _cache[key] = _build(pieces)
    nc = _cache[key]

    s_p, p_p, t_p = s[perm], p[perm], times[perm]

    def bt(x):
        return np.ascontiguousarray(x, dtype=BF16)

    x1_in = bt(np.asarray(S1)[s_p].T)
    x2_in = bt(np.asarray(S2)[s_p].T)
    ts_in = bt(np.asarray(T_S)[t_p].T)
    to_in = bt(np.asarray(T_O)[t_p].T)
    pp1 = np.asarray(P1)[p_p]                       # [B, RD]
    pp2 = np.asarray(P2)[p_p]
    G1 = np.asarray(G1); G2 = np.asarray(G2)
    O1 = np.asarray(O1); O2 = np.asarray(O2)
    bnp = np.stack([g11, b11, g12, b12, g21, b21, g22, b22], axis=1).astype(np.float32)
    bnp = np.ascontiguousarray(bnp)

    in_maps = []
    for k in range(NCORES):
        rs = slice(RS * k, RS * (k + 1))
        vs = slice(ES * k, ES * (k + 1))
        g1k = bt(G1[rs].reshape(RS, 2, 100, ED).transpose(2, 0, 1, 3))
        g2k = bt(G2[:, rs].reshape(C, RS, 2, 100, ED).transpose(1, 2, 3, 0, 4))
        pb1 = bt(pp1[:, rs].T)
        pb2 = bt(pp2[:, rs].T)
        o1t = bt(O1[vs].T)   # [200, ES]
        o2t = bt(O2[vs].T)
        in_maps.append({
            "x1_in": x1_in, "x2_in": x2_in, "ts_in": ts_in, "to_in": to_in,
            "g1_in": g1k, "g2_in": g2k, "pb1_in": pb1, "pb2_in": pb2,
            "oc0_in": np.ascontiguousarray(o1t[0:128]),
            "oc1_in": np.ascontiguousarray(o1t[128:200]),
            "oc2_in": np.ascontiguousarray(o2t[0:128]),
            "oc3_in": np.ascontiguousarray(o2t[128:200]),
            "bnp_in": bnp,
        })

    res = run_bass_kernel_spmd(nc, in_maps, core_ids=list(range(NCORES)))

    out_sorted = np.concatenate([res.results[k]["out"] for k in range(NCORES)], axis=1)
    out = np.empty_like(out_sorted)
    out[perm] = out_sorted
    return out
